# revision 57
# baseline (speedup 1.0000x reference)
"""BertSelfAttention forward on 8 Trainium2 NeuronCores (Bass/Tile).

Problem: B=2, S=2048, HIDDEN=1024, 16 heads x head_dim 64, fp32 I/O.

Sharding: core c handles batch b = c//4 and head-group g = c%4
(heads 4g..4g+4 == hidden columns 256g..256g+256). Attention is
embarrassingly parallel per (batch, head): no collectives; each core
computes a disjoint [S, 256] slice of the output.

Host-side layout preparation (same as prior version): hsT pre-transposed
bf16 [1024, 2048]; W q/k split by 128-col half and Wv packed in SBUF
tile layout; bv and the softmax division live on the host.

Device program v2 — built around the ScalarE exp wall (64 kt-slots x
2292ns = 147us of ACTIVATE at N=1024):
  1. Scores row-tiled as before (two heads in PE rows 0-63 / 64-127,
     K=64), but the scores PSUM is a 2-slot rotation (4 banks), so the
     scores matmuls for exp e+2 run during exp e+1 and the exp stream
     never waits on the psum handoff (the old single-buffer ping-pong
     cost ~390ns/kt plus pile-ups).
  2. ctx is COLUMN-TILED: v per head is [128k, 64] (M=64), so two
     heads' ctx matmuls run concurrently on PE column halves
     (tile_position (0,0)/(0,64)), halving ctx stream time vs the old
     M=65 full-array version. ctx psum is one [128, 512] tile per
     j-chunk (partitions 0-63 = even head, 64-127 = odd head), 2 banks.
  3. The softmax denominator (old 65th v-row trick) moves to VectorE:
     dacc[g,hh] += pt (bf16 tensor_add, 2x mode) accumulated over the
     16 key tiles, shipped to HBM as [128, 1024] partials; the host
     does the exact 128-partition sum. Zero PE/PSUM cost.
  4. PSUM: scores 2x[128,1024] (4 banks) + proj/v 2x[128,512] (psQ) +
     ctx 2x[128,512] (psC) = exactly 16KB. No mid-kernel pool switch.
  5. Work queues: pe_slow (proj/v pieces, FIFO: v st0-7, q/k dc0-scg1,
     v st8-15, q/k dc1) and pe_fast (ctx pieces, enqueued as their
     probs tiles are exp'd, trailing ~2 kt) pumped per kt-slot with a
     PE budget; den adds go on a DVE queue with its own budget. Fast
     pops force-flush slow v pieces first so the in-order PE never
     waits on work behind it in its own stream.
  6. DMA strictly in consumption order (W dc0 + hsT half0 first -> wv
     -> half1 -> W dc1) over 3 queues; the first exp needs only W dc0
     + half0 (~2.5MB), so the exp stream starts ~15us in and is DMA-
     gapless after that.
"""

import sys
from collections import deque

for _p in ("/opt/trn_rl_repo",):
    if _p not in sys.path:
        sys.path.insert(0, _p)

import ml_dtypes
import numpy as np

import concourse.bass as bass  # noqa: F401
import concourse.mybir as mybir
import concourse.tile as tile
from concourse import bacc
from concourse.bass_utils import run_bass_kernel_spmd

B, S, HID = 2, 2048, 1024
NH, HD = 16, 64
N_CORES = 8
GH = 4  # heads per core
GD = GH * HD  # 256
P = 128
ST = S // P  # 16 key tiles
HC = HID // P  # 8 hidden chunks
QC = 4  # q chunks of 512
QW = S // QC  # 512
SH = S // 2  # 1024 (hsT half width)
F32 = mybir.dt.float32
BF16 = mybir.dt.bfloat16
EXP = mybir.ActivationFunctionType.Exp
PBUFS = 20  # probs tiles in flight per hh tag

_CACHE = {}


def _build_nc():
    nc = bacc.Bacc("TRN2", target_bir_lowering=False, debug=False, num_devices=N_CORES)

    # chunk-major so every hsT tile DMA is one contiguous 256KB block
    hsT_d = nc.dram_tensor("hsT", [HC, 2, P, SH], BF16, kind="ExternalInput").ap()
    wq_d = [
        nc.dram_tensor(f"wq{dc}", [P, HC, P], BF16, kind="ExternalInput").ap()
        for dc in range(2)
    ]
    wk_d = [
        nc.dram_tensor(f"wk{dc}", [P, HC, P], BF16, kind="ExternalInput").ap()
        for dc in range(2)
    ]
    wv_d = nc.dram_tensor("wv", [P, HC, GD], BF16, kind="ExternalInput").ap()
    # packed per-partition smalls: cols 0-1 bq(dc), 2-3 bk(dc), 4-19 mask(kt)
    small_d = nc.dram_tensor("small", [P, 4 + ST], F32, kind="ExternalInput").ap()
    # ctx numerators, tile-major: [pair, qcg, j, hh*64+d, q] (contiguous DMAs)
    yt_d = nc.dram_tensor("yt", [2, 2, 2, P, QW], BF16, kind="ExternalOutput").ap()
    # denominator partials, tile-major; host sums over the 128 partitions
    dacc_d = nc.dram_tensor("dacc", [4, 2, P, 2 * QW], BF16, kind="ExternalOutput").ap()

    with tile.TileContext(nc) as tc:
        with (
            tc.tile_pool(name="const", bufs=1) as constp,
            tc.tile_pool(name="big", bufs=1) as bigp,
            tc.tile_pool(name="probs", bufs=1) as probsp,
            tc.tile_pool(name="dap", bufs=1) as daccp,
            tc.tile_pool(name="outp", bufs=1) as outp,
            tc.tile_pool(name="psS", bufs=1, space="PSUM") as psS,
            tc.tile_pool(name="psQ", bufs=1, space="PSUM") as psQ,
            tc.tile_pool(name="psC", bufs=1, space="PSUM") as psC,
        ):
            wq_sb = [constp.tile([P, HC, P], BF16, name=f"wq{dc}") for dc in range(2)]
            wk_sb = [constp.tile([P, HC, P], BF16, name=f"wk{dc}") for dc in range(2)]
            wv_sb = constp.tile([P, HC, GD], BF16)
            small_sb = constp.tile([P, 4 + ST], F32)
            hsTt = [
                [bigp.tile([P, SH], BF16, name=f"hsT{hc}_{h}") for h in range(2)]
                for hc in range(HC)
            ]
            qTc = [[None] * QC for _ in range(2)]
            kTc = [[None] * QC for _ in range(2)]
            for dc in range(2):
                for sc in range(QC):
                    qTc[dc][sc] = bigp.tile([P, QW], BF16, name=f"qT{dc}_{sc}")
                    kTc[dc][sc] = bigp.tile([P, QW], BF16, name=f"kT{dc}_{sc}")
            v_sb = bigp.tile([P, ST, GH, HD], BF16)

            # ---- DMA issue: 5 rings; half0 spread so the inline proj can
            # consume in arrival order (single-item tensor/vector rings land
            # their chunks first; W dc0 heads the sync/scalar rings) ----
            def hst_dma(eng, hc, h):
                eng.dma_start(hsTt[hc][h][:], hsT_d[hc, h])

            nc.gpsimd.dma_start(small_sb[:], small_d[:])
            nc.sync.dma_start(wk_sb[0][:], wk_d[0][:])
            nc.scalar.dma_start(wq_sb[0][:], wq_d[0][:])
            for hc, eng in (  # half 0, rings balanced; arrivals ~(2,0,1,5,3,4,6,7)
                (2, nc.gpsimd), (0, nc.sync), (1, nc.scalar), (5, nc.gpsimd),
                (3, nc.sync), (4, nc.scalar), (6, nc.sync), (7, nc.scalar),
            ):
                hst_dma(eng, hc, 0)
            INLINE_HC_ORDER = (2, 0, 1, 5, 3, 4, 6, 7)
            nc.gpsimd.dma_start(wv_sb[:], wv_d[:])  # needed by v st0-7 (~17us)
            for hc, eng in (  # half 1: scg1 proj (hc order) + v st8-15
                (0, nc.sync), (1, nc.scalar), (2, nc.gpsimd), (3, nc.sync),
                (4, nc.scalar), (5, nc.gpsimd), (6, nc.sync), (7, nc.scalar),
            ):
                hst_dma(eng, hc, 1)
            nc.scalar.dma_start(wq_sb[1][:], wq_d[1][:])
            nc.sync.dma_start(wk_sb[1][:], wk_d[1][:])

            # ---- PE p-state ramp + exp table preload ----
            pewarm = constp.tile([P, QW], BF16)
            nc.vector.memset(pewarm[:], 0.0)
            for i in range(12):
                if i % 2 == 0:
                    pw = psQ.tile([P, QW], F32, tag="ps", bufs=2, name="pw")
                else:
                    pw = psC.tile([P, QW], F32, tag=f"c{(i // 2) % 2}", bufs=1,
                                  name="pw")
                nc.tensor.matmul(
                    pw[:], lhsT=pewarm[:, 0:P], rhs=pewarm[:], start=True, stop=True
                )
            warm = constp.tile([P, 1], F32)
            warm2 = constp.tile([P, 1], F32)
            nc.vector.memset(warm[:], 0.0)
            nc.scalar.activation(warm2[:], warm[:], EXP)

            # ---- work queues ----
            pe_slow = deque()  # proj/v pieces, dependency-ordered
            pe_fast = deque()  # ctx pieces, trail the exp stream
            dve_work = deque()  # den accumulation pieces
            v_issued = [0]  # count of v st whose BOTH halves are issued
            pe_bank = [0.0]
            dve_bank = [0.0]
            cur_slot = [0]  # global kt-slot counter

            def pump(pe_budget=1.45, dve_budget=2.05):
                # Slot-gating: a ctx piece for slot s pops at s+1, a den piece
                # at s+2 — popping earlier makes the piece's exp-semaphore
                # wait head-block the in-order PE/DVE streams (delaying the
                # next scores / the critical evacs behind it).
                pe_bank[0] = min(pe_bank[0] + pe_budget, 3.0 * pe_budget)
                while True:
                    if pe_slow and pe_bank[0] >= getattr(pe_slow[0], "_cost", 0.5):
                        sfn = pe_slow.popleft()
                        pe_bank[0] -= getattr(sfn, "_cost", 0.5)
                        sfn()
                        continue
                    if (
                        pe_fast
                        and pe_fast[0]._slot < cur_slot[0]
                        and pe_bank[0] >= getattr(pe_fast[0], "_cost", 0.5)
                    ):
                        fn = pe_fast[0]
                        need_v = getattr(fn, "_need_v", -1)
                        while v_issued[0] <= need_v and pe_slow:
                            sfn = pe_slow.popleft()
                            pe_bank[0] -= getattr(sfn, "_cost", 0.5)
                            sfn()
                        pe_fast.popleft()
                        pe_bank[0] -= getattr(fn, "_cost", 0.5)
                        fn()
                        continue
                    break
                dve_bank[0] = min(dve_bank[0] + dve_budget, 2.0 * dve_budget)
                while (
                    dve_work
                    and dve_work[0]._slot + getattr(dve_work[0], "_gate", 2)
                    <= cur_slot[0]
                    and dve_bank[0] >= getattr(dve_work[0], "_cost", 1.2)
                ):
                    dfn = dve_work.popleft()
                    dve_bank[0] -= getattr(dfn, "_cost", 1.2)
                    dfn()

            def force_deadlines():
                # issue (FIFO) everything whose deadline has arrived, so data
                # writers are always issued before their scores readers
                while pe_slow and min(
                    getattr(p, "_dl", 999) for p in pe_slow
                ) <= cur_slot[0]:
                    pe_slow.popleft()()

            def rhs_for(hc, sc):  # hsT [d-slab, 512 seq] slice
                return hsTt[hc][sc // 2][:, (sc % 2) * QW : (sc % 2 + 1) * QW]

            # ---- projection pieces (psQ tag "ps", bufs=2 rotation) ----
            proj_state = {}

            def proj_eighth(dst_chunks, bias_col, wt, dc, scg, hc):
                scs = (2 * scg, 2 * scg + 1)
                key = (bias_col, dc, scg)
                if hc == 0:
                    proj_state[key] = [
                        psQ.tile([P, QW], F32, tag="ps", bufs=2, name=f"pp{i}")
                        for i in range(2)
                    ]
                pps = proj_state[key]
                for i, sc in enumerate(scs):
                    nc.tensor.matmul(
                        pps[i][:],
                        lhsT=wt[:, hc, :],
                        rhs=rhs_for(hc, sc),
                        start=(hc == 0),
                        stop=(hc == HC - 1),
                    )
                if hc == HC - 1:
                    for i, sc in enumerate(scs):
                        nc.vector.tensor_scalar_add(
                            out=dst_chunks[sc][:],
                            in0=pps[i][:],
                            scalar1=small_sb[:, bias_col : bias_col + 1],
                        )
                    del proj_state[key]

            v_state = {}

            def v_half(st, half):
                if half == 0:
                    v_state[st] = psQ.tile([P, GD], F32, tag="ps", bufs=2, name="pv")
                pv = v_state[st]
                for hc in range(4 * half, 4 * half + 4):
                    nc.tensor.matmul(
                        pv[:],
                        lhsT=hsTt[hc][st // 8][:, (st % 8) * P : (st % 8 + 1) * P],
                        rhs=wv_sb[:, hc, :],
                        start=(hc == 0),
                        stop=(hc == HC - 1),
                    )
                if half == 1:
                    nc.vector.tensor_copy(
                        v_sb[:, st, :, :],
                        pv[:].rearrange("p (h d) -> p h d", d=HD),
                    )
                    del v_state[st]
                    v_issued[0] += 1

            def add_v(st):
                for half in range(2):
                    f = lambda st=st, half=half: v_half(st, half)
                    f._cost = 0.45
                    pe_slow.append(f)

            def add_proj(dst, bias_col, wt, dc, scg, dl0=999):
                for hc in range(HC):
                    f = lambda hc=hc: proj_eighth(dst, bias_col, wt, dc, scg, hc)
                    f._cost = 0.45
                    f._dl = dl0 + hc if dl0 != 999 else 999
                    pe_slow.append(f)

            # ---- ctx (column-tiled, two heads concurrent) + den pieces ----
            pts = [{0: [], 1: []} for _ in range(4)]
            cps = [None] * 4
            daccs = [[None, None] for _ in range(4)]

            def ctx_kt_j(g, pair, qcg, kt, j):
                if kt == 0 and j == 0:
                    cps[g] = [
                        psC.tile([P, QW], F32, tag=f"c{jj}", bufs=1, name=f"cp{jj}")
                        for jj in range(2)
                    ]
                cp = cps[g][j]
                for hh, tp, prt in (
                    (0, (0, 0), slice(0, 64)),
                    (1, (0, 64), slice(64, 128)),
                ):
                    nc.tensor.matmul(
                        cp[prt, :],
                        lhsT=v_sb[:, kt, 2 * pair + hh, :],
                        rhs=pts[g][hh][kt][:, j * QW : (j + 1) * QW],
                        start=(kt == 0),
                        stop=(kt == ST - 1),
                        tile_position=tp,
                        skip_group_check=True,
                    )
                if kt == ST - 1:
                    # bf16 numerators halve the output DMA; host divides in f32
                    ctxs = outp.tile([P, QW], BF16, tag="ctxs", bufs=4)
                    nc.vector.tensor_copy(ctxs[:], cp[:])
                    # scalar-ring descriptors would interrupt the exp stream;
                    # only the last group's outputs (post-stream) may use it
                    qdma = nc.scalar if (g == 3 and j == 1) else nc.sync
                    qdma.dma_start(yt_d[pair, qcg, j], ctxs[:])

            def den_kt(g, kt):
                for hh in range(2):
                    pt = pts[g][hh][kt]
                    if kt == 0:
                        daccs[g][hh] = daccp.tile(
                            [P, 2 * QW], BF16, tag=f"d{hh}", bufs=2, name=f"da{hh}"
                        )
                        nc.vector.tensor_copy(daccs[g][hh][:], pt[:])
                    else:
                        nc.vector.tensor_add(daccs[g][hh][:], daccs[g][hh][:], pt[:])
                if kt == ST - 1:
                    for hh in range(2):
                        qdma = nc.scalar if (g == 3 and hh == 1) else nc.gpsimd
                        qdma.dma_start(dacc_d[g, hh], daccs[g][hh][:])

            # ---- inline interleaved q/k dc0 scg0, paced to hsT arrivals ----
            # (k borrows psC's banks; ctx g0 starts well after the evac)
            # kt0 of emit 0 needs only qT sc0+sc1 and kT sc0: inline those
            # three chains; kT sc1 (needed from kt4, ~9us later) goes on the
            # queue. Evacs split DVE / ScalarE-Copy (Copy is in the exp table
            # set and the exp stream hasn't started yet).
            ppq = [
                psQ.tile([P, QW], F32, tag="ps", bufs=2, name=f"ppq{i}")
                for i in range(2)
            ]
            ppk0 = psC.tile([P, QW], F32, tag="c0", bufs=1, name="ppk0")
            for n, hc in enumerate(INLINE_HC_ORDER):
                for i, sc in ((0, 0), (1, 1)):
                    nc.tensor.matmul(
                        ppq[i][:], lhsT=wq_sb[0][:, hc, :], rhs=rhs_for(hc, sc),
                        start=(n == 0), stop=(n == HC - 1),
                    )
                nc.tensor.matmul(
                    ppk0[:], lhsT=wk_sb[0][:, hc, :], rhs=rhs_for(hc, 0),
                    start=(n == 0), stop=(n == HC - 1),
                )
            nc.vector.tensor_scalar_add(
                out=qTc[0][0][:], in0=ppq[0][:], scalar1=small_sb[:, 0:1]
            )
            nc.scalar.activation(
                kTc[0][0][:], ppk0[:], mybir.ActivationFunctionType.Identity,
                bias=small_sb[:, 2:3],
            )
            nc.vector.tensor_scalar_add(
                out=qTc[0][1][:], in0=ppq[1][:], scalar1=small_sb[:, 0:1]
            )

            # deferred kT dc0 sc1 as queue pieces (one matmul each)
            ksc1_state = {}

            def ksc1_piece(n):
                hc = INLINE_HC_ORDER[n]
                if n == 0:
                    ksc1_state["pp"] = psC.tile(
                        [P, QW], F32, tag="c1", bufs=1, name="ppk1"
                    )
                nc.tensor.matmul(
                    ksc1_state["pp"][:], lhsT=wk_sb[0][:, hc, :],
                    rhs=rhs_for(hc, 1), start=(n == 0), stop=(n == HC - 1),
                )
                if n == HC - 1:
                    nc.vector.tensor_scalar_add(
                        out=kTc[0][1][:], in0=ksc1_state["pp"][:],
                        scalar1=small_sb[:, 2:3],
                    )

            for n in range(HC):
                f = lambda n=n: ksc1_piece(n)
                f._cost = 0.25
                f._dl = max(1, n - 3)  # all in by slot 4 (kt4 reads kT sc1)
                pe_slow.append(f)

            # slow queue, deadline-ordered: kT0-scg1 by kt8 (slot 8), qT0-scg1
            # by emit 1 (slot 16), dc1 scg0 by emit 2 (slot 32), dc1 scg1 by
            # kt8 of emit 2; v interleaved (ctx-guard pulls it if needed).
            add_proj(kTc[0], 2, wk_sb[0], 0, 1, dl0=1)  # in by kt8 of emit 0
            for st in range(8):
                add_v(st)
            add_proj(qTc[0], 0, wq_sb[0], 0, 1, dl0=8)  # in by emit 1 (slot 16)
            for st in range(8, ST):
                add_v(st)
            add_proj(kTc[1], 3, wk_sb[1], 1, 0, dl0=16)  # in by emit 2 (slot 32)
            add_proj(qTc[1], 1, wq_sb[1], 1, 0, dl0=24)

            # ---- scores + exp emitter; ctx/den trail via the queues ----
            exp_ctr = [0]

            def scores_emit(pair, qcg, g, pe_budget=1.5):
                q0, q1 = 2 * qcg, 2 * qcg + 1
                for kt in range(ST):
                    force_deadlines()
                    sc, kk = divmod(kt, 4)
                    for hh, rows, tp in (
                        (0, slice(0, 64), (0, 0)),
                        (1, slice(64, 128), (64, 0)),
                    ):
                        slot = psS.tile(
                            [P, 2 * QW], F32, tag=f"s{exp_ctr[0] % 2}", bufs=1
                        )
                        exp_ctr[0] += 1
                        for j, qq in ((0, q0), (1, q1)):
                            nc.tensor.matmul(
                                slot[:, j * QW : (j + 1) * QW],
                                lhsT=kTc[pair][sc][rows, kk * P : (kk + 1) * P],
                                rhs=qTc[pair][qq][rows, :],
                                start=True,
                                stop=True,
                                tile_position=tp,
                            )
                        pt = probsp.tile(
                            [P, 2 * QW], BF16, tag=f"p{hh}", bufs=PBUFS,
                            name=f"pt{hh}_{kt}",
                        )
                        nc.scalar.activation(
                            pt[:],
                            slot[:],
                            EXP,
                            bias=small_sb[:, 4 + kt : 5 + kt],
                            scale=0.125,
                        )
                        pts[g][hh].append(pt)
                    for j in range(2):
                        f = lambda g=g, pair=pair, qcg=qcg, kt=kt, j=j: ctx_kt_j(
                            g, pair, qcg, kt, j
                        )
                        f._cost = 0.25  # the two col-tiled matmuls overlap
                        f._need_v = kt
                        f._slot = cur_slot[0]
                        pe_fast.append(f)
                    d = lambda g=g, kt=kt: den_kt(g, kt)
                    d._cost = 0.7 if kt == 0 else 1.2
                    d._slot = cur_slot[0]
                    # groups 0-1 trail loosely (the pt pool allows ~6 slots),
                    # shifting their DVE adds out of the congested first half;
                    # groups 2-3 stay tight so the tail doesn't grow
                    d._gate = 6 if g < 2 else 2
                    dve_work.append(d)
                    # pump AFTER this slot's scores/exp so queue pieces never
                    # sit ahead of scores in the in-order PE stream
                    pump(pe_budget=pe_budget)
                    cur_slot[0] += 1

            add_proj(kTc[1], 3, wk_sb[1], 1, 1, dl0=32)  # in by kt8 of emit 2
            add_proj(qTc[1], 1, wq_sb[1], 1, 1, dl0=40)  # in by emit 3
            scores_emit(0, 0, 0, pe_budget=1.45)
            scores_emit(0, 1, 1, pe_budget=1.55)
            scores_emit(1, 0, 2, pe_budget=1.5)
            scores_emit(1, 1, 3, pe_budget=1.5)

            while pe_fast or pe_slow or dve_work:
                pe_bank[0] = 10.0
                dve_bank[0] = 10.0
                cur_slot[0] += 1
                pump()
    nc.compile()
    return nc


def _make_in_maps(hidden_states, attention_mask, Wq, bq, Wk, bk, Wv, bv):
    min_val = np.finfo(np.float32).min
    # [1024, 2048] -> [hc, half, p, sh]: each SBUF tile contiguous in HBM
    hsT_by_b = [
        np.ascontiguousarray(
            hidden_states[b].T.reshape(HC, P, 2, SH).transpose(0, 2, 1, 3)
        ).astype(ml_dtypes.bfloat16)
        for b in range(B)
    ]
    mask_by_b = [
        np.ascontiguousarray(
            ((1.0 - attention_mask[b]) * min_val).astype(np.float32).reshape(ST, P).T
        )
        for b in range(B)
    ]

    def packw(W, sl):
        # [1024, 256] -> [128, 8, 256] so SBUF partition p holds rows p, 128+p, ...
        return np.ascontiguousarray(
            W[:, sl].reshape(HC, P, GD).transpose(1, 0, 2)
        ).astype(ml_dtypes.bfloat16)

    def packw_dc(W, sl, dc):
        return np.ascontiguousarray(packw(W, sl)[:, :, dc * P : (dc + 1) * P])

    in_maps = []
    for c in range(N_CORES):
        b, g = divmod(c, N_CORES // B)
        sl = slice(GD * g, GD * (g + 1))
        small = np.concatenate(
            [bq[sl].reshape(2, P).T, bk[sl].reshape(2, P).T, mask_by_b[b]], axis=1
        ).astype(np.float32)
        in_maps.append(
            {
                "hsT": hsT_by_b[b],
                "wq0": packw_dc(Wq, sl, 0),
                "wq1": packw_dc(Wq, sl, 1),
                "wk0": packw_dc(Wk, sl, 0),
                "wk1": packw_dc(Wk, sl, 1),
                "wv": packw(Wv, sl),
                "small": np.ascontiguousarray(small),
            }
        )
    return in_maps


def _unpack_out(res, bv):
    out = np.empty((B, S, HID), dtype=np.float32)
    for c in range(N_CORES):
        b, gc = divmod(c, N_CORES // B)
        ytc = np.asarray(res.results[c]["yt"], dtype=np.float32)  # [2,2,2,128,512]
        dac = np.asarray(res.results[c]["dacc"], dtype=np.float32)  # [4,2,1024... ]
        for g in range(4):
            pair, qcg = divmod(g, 2)
            for hh in range(2):
                hl = 2 * pair + hh  # local head 0..3
                # den partial [128, 2, 512] -> exact sum over partitions
                den = dac[g, hh].reshape(P, 2, QW).sum(axis=0)  # [j, 512]
                cols = slice(GD * gc + HD * hl, GD * gc + HD * (hl + 1))
                for j in range(2):
                    blk = ytc[pair, qcg, j, hh * HD : (hh + 1) * HD]  # [64, 512]
                    qs = slice((2 * qcg + j) * QW, (2 * qcg + j + 1) * QW)
                    out[b, qs, cols] = (blk / den[j]).T + bv[cols]
    return out


def kernel(hidden_states, attention_mask, Wq, bq, Wk, bk, Wv, bv):
    hidden_states = np.asarray(hidden_states, dtype=np.float32)
    attention_mask = np.asarray(attention_mask, dtype=np.float32)
    Wq, Wk, Wv = (np.asarray(a, dtype=np.float32) for a in (Wq, Wk, Wv))
    bq, bk, bv = (np.asarray(a, dtype=np.float32) for a in (bq, bk, bv))

    if "nc" not in _CACHE:
        _CACHE["nc"] = _build_nc()
    nc = _CACHE["nc"]

    in_maps = _make_in_maps(hidden_states, attention_mask, Wq, bq, Wk, bk, Wv, bv)
    res = run_bass_kernel_spmd(nc, in_maps, list(range(N_CORES)))
    return _unpack_out(res, bv)
